# revision 18
# baseline (speedup 1.0000x reference)
"""Trainium2 Bass kernel for the temporal/spatial adapter transformer block.

Sharding: data-parallel over the video batch B=8 -> 1 video (16 frames) per
NeuronCore; all weights replicated. No collectives.

Per-core layout strategy:
  - token-major fp32 residual stream (LayerNorm stats via bn_stats,
    per-partition mean/rstd scalars),
  - feature-major bf16 compute stream for all matmul chains,
  - PE transposes only on bf16 tensors (1 cycle/row),
  - attention computed with transposed scores (S^T = k^T . q) so softmax
    normalization reduces over the partition dim via ones-matmuls; the
    1/sum normalization is deferred onto o via a PE broadcast.
"""

import os
import sys

import numpy as np
import ml_dtypes

try:
    import concourse.bass  # noqa: F401
except ImportError:  # concourse ships with the container, not on sys.path
    for p in ("/opt/trn_rl_repo", "/root/.axon_site/_ro/trn_rl_repo"):
        if p not in sys.path:
            sys.path.insert(0, p)

import concourse.bass as bass
import concourse.mybir as mybir
import concourse.tile as tile
from concourse import bacc
from concourse.bass_utils import run_bass_kernel_spmd

BF = mybir.dt.bfloat16
F32 = mybir.dt.float32
AF = mybir.ActivationFunctionType
OP = mybir.AluOpType

P = 128
NSEQ = 197          # tokens per frame/sequence
D = 768
DK = D // P         # 6
H = 12
HD = 64
BOT = 192
HID = 4 * D         # 3072
HK = HID // P       # 24
EPS = 1e-5
T = 16              # frames per video
TT = 8              # temporal frames
NCORES = 8
TAU = 2 * NSEQ      # tokens per pair = 394
ROWS = T * NSEQ     # 3152 rows per core

GELU_C = 0.044715
GELU_S = 0.7978845608028654  # sqrt(2/pi)
QK_SCALE = HD ** -0.5

ADAPTERS = ("tab", "sa", "ta", "sm", "tm")

bf16 = ml_dtypes.bfloat16


# ----------------------------------------------------------------------------
# host-side weight preprocessing (shared by all cores)
# ----------------------------------------------------------------------------

def preprocess_weights(inp):
    """Build the per-core constant input arrays (already in SBUF layout)."""
    w = {}

    def fm(mat):  # [out, in] -> lhsT layout [128, in//128, out]
        o, i = mat.shape
        return np.ascontiguousarray(
            mat.T.reshape(i // P, P, o).transpose(1, 0, 2)).astype(bf16)

    qkv = np.asarray(inp["qkv_w"], np.float32).copy()
    qkv[:D] *= QK_SCALE  # fold attention scale into q
    w["wqkv"] = fm(qkv)                                   # [128, 6, 2304]

    w["wproj"] = fm(np.asarray(inp["proj_w"], np.float32))  # [128, 6, 768]
    w["bproj"] = np.asarray(inp["proj_b"], np.float32).reshape(DK, P).T.copy()

    a = fm(np.asarray(inp["fc1_w"], np.float32))            # [128, 6, 3072]
    w["wfc1"] = np.ascontiguousarray(
        a.reshape(P, DK, HK, P).transpose(2, 0, 1, 3).reshape(HK, P, DK * P))
    b1 = np.asarray(inp["fc1_b"], np.float32)
    w["bfc1s"] = b1.reshape(HK, P).T.copy()
    a = fm(np.asarray(inp["fc2_w"], np.float32))            # [128, 24, 768]
    w["wfc2"] = np.ascontiguousarray(
        a.reshape(P, HK, DK, P).transpose(2, 0, 1, 3).reshape(DK, P, HK * P))
    w["bfc2"] = np.asarray(inp["fc2_b"], np.float32).reshape(DK, P).T.copy()

    for ad in ADAPTERS:
        dw = np.asarray(inp[ad + "_dw"], np.float32)        # [192, 768]
        db = np.asarray(inp[ad + "_db"], np.float32)        # [192]
        uw = np.asarray(inp[ad + "_uw"], np.float32)        # [768, 192]
        ub = np.asarray(inp[ad + "_ub"], np.float32)        # [768]
        w["w%sd" % ad] = fm(dw)                             # [128, 6, 192]
        bd = np.zeros((P, 2), np.float32)
        bd[:, 0] = db[:P]
        bd[:64, 1] = db[P:]
        w["b%sd" % ad] = bd
        # up: lhsT [192, 768] -> [128, 2, 768], chunk1 rows 64:128 zero
        up = np.zeros((2 * P, D), np.float32)
        up[:BOT] = uw.T
        w["w%su" % ad] = up.reshape(2, P, D).transpose(1, 0, 2).astype(bf16)
        w["b%su" % ad] = ub.reshape(DK, P).T.copy()

    for nm, key in (("g1", "n1_g"), ("b1", "n1_b"), ("g2", "n2_g"), ("b2", "n2_b")):
        w[nm] = np.asarray(inp[key], np.float32).reshape(DK, P).T.copy()

    w["ident"] = np.eye(P, dtype=bf16)
    w["ones"] = np.ones((P, P), dtype=bf16)
    w["epsc"] = np.full((P, 1), EPS, np.float32)
    return w


STREAMED_SPECS = [
    ("wfc1", [HK, P, DK * P], BF),
    ("wfc2", [DK, P, HK * P], BF),
]

WEIGHT_SPECS = [
    ("wqkv", [P, DK, 3 * D], BF),
    ("wproj", [P, DK, D], BF), ("bproj", [P, DK], F32),
    ("bfc1s", [P, HK], F32),
    ("bfc2", [P, DK], F32),
    ("g1", [P, DK], F32), ("b1", [P, DK], F32),
    ("g2", [P, DK], F32), ("b2", [P, DK], F32),
    ("ident", [P, P], BF), ("ones", [P, P], BF), ("epsc", [P, 1], F32),
] + [
    it for ad in ADAPTERS for it in [
        ("w%sd" % ad, [P, DK, BOT], BF),
        ("b%sd" % ad, [P, 2], F32),
        ("w%su" % ad, [P, 2, D], BF),
        ("b%su" % ad, [P, DK], F32),
    ]
]


# ----------------------------------------------------------------------------
# program emission
# ----------------------------------------------------------------------------

# token tiles of a pair: (row_offset_within_pair, nrows, fm_col_offset)
PAIR_TILES = [(0, P, 0), (P, NSEQ - P, P),
              (NSEQ, P, NSEQ), (NSEQ + P, NSEQ - P, NSEQ + P)]


class Ctx:
    pass


def make_pools(ctx, tc, es):
    def pool(name, bufs):
        return es.enter_context(tc.tile_pool(name=name, bufs=bufs))

    def ppool(name, bufs):
        return es.enter_context(tc.tile_pool(name=name, bufs=bufs, space="PSUM"))

    ctx.weights = pool("weights", 1)
    ctx.xres = pool("xres", 8)       # token-major f32 residual stream
    ctx.small = pool("small", 8)     # bn stats etc.
    ctx.xn = pool("xn", 2)           # token-major bf16 LN output
    ctx.fmA = pool("fmA", 2)         # xnT / xn2T
    ctx.fmB = pool("fmB", 2)         # tab-out / attnT / mlpT (matmul inputs)
    ctx.fmC = pool("fmC", 2)         # delta tiles
    ctx.qk = pool("qk", 2)           # q,k feature-major
    ctx.vt = pool("vt", 4)           # v token-major
    ctx.oT = pool("oT", 2)
    ctx.sa = pool("sa", 2)           # saT / smT
    ctx.ae = pool("ae", 3)           # exp'd scores bf16
    ctx.rr = pool("rr", 3)           # softmax recip (f32+bf16 tiny)
    ctx.rbs = pool("rbs", 1)         # broadcast recip SBUF f32
    ctx.g2 = pool("g2", 1)           # mlp gelu output
    ctx.wf1 = pool("wf1", 3)         # streamed fc1 weight tiles
    ctx.wf2 = pool("wf2", 2)         # streamed fc2 weight tiles
    ctx.u = pool("u", 2)             # adapter gelu outputs bf16

    ctx.pmm = ppool("pmm", 2)        # dense matmul outputs [128, TAU]
    ctx.ptp = ppool("ptp", 2)        # transposes [128, 128]
    ctx.psT = ppool("psT", 1)        # scores
    ctx.prb = ppool("prb", 1)        # softmax sum + broadcast
    ctx.po = ppool("po", 2)          # attention o


def load_weights(ctx, nc, d):
    ctx.W = {}
    for name, shape, dt in WEIGHT_SPECS:
        t = ctx.weights.tile(shape, dt, tag=name)
        nc.sync.dma_start(t[:], d[name][:])
        ctx.W[name] = t


def emit_ln(ctx, nc, xts, tiles, gname, bname):
    """token-major LN on xts (f32) -> feature-major bf16 [128, DK, TAU]."""
    W = ctx.W
    # batched stats: one sqrt + one reciprocal for all 4 token tiles
    mv4 = ctx.small.tile([P, 4, 2], F32, tag="bnmv4")
    for i, (r0, pi, co) in enumerate(tiles):
        xt = xts[i]
        st = ctx.small.tile([P, 2, 6], F32, tag="bnst")
        nc.vector.bn_stats(st[:pi, 0, :], xt[:pi, 0:D // 2])
        nc.vector.bn_stats(st[:pi, 1, :], xt[:pi, D // 2:D])
        nc.vector.bn_aggr(mv4[:pi, i, :], st[:pi])
    sd4 = ctx.small.tile([P, 4], F32, tag="sd4")
    nc.scalar.activation(sd4[:], mv4[:, :, 1], AF.Sqrt, bias=W["epsc"][:, 0:1])
    rstd4 = ctx.small.tile([P, 4], F32, tag="rstd4")
    nc.vector.reciprocal(rstd4[:], sd4[:])
    xns = []
    for i, (r0, pi, co) in enumerate(tiles):
        xn = ctx.xn.tile([P, D], BF, tag="xn")
        nc.vector.tensor_scalar(xn[:pi], xts[i][:pi], mv4[:pi, i, 0:1],
                                rstd4[:pi, i:i + 1],
                                op0=OP.subtract, op1=OP.mult)
        xns.append(xn)
    xnT = ctx.fmA.tile([P, DK, TAU], BF, tag="xnT")
    for i, (r0, pi, co) in enumerate(tiles):
        for j in range(DK):
            tp = ctx.ptp.tile([P, 1024], BF, tag="tp", name="tp")
            tp = tp[:, :P]
            nc.tensor.transpose(tp[:P, :pi], xns[i][:pi, j * P:(j + 1) * P],
                                W["ident"][:pi, :pi])
            nc.vector.tensor_scalar(xnT[:, j, co:co + pi], tp[:, :pi],
                                    W[gname][:, j:j + 1], W[bname][:, j:j + 1],
                                    op0=OP.mult, op1=OP.add)
    return xnT


def emit_adapter(ctx, nc, ad, inT, combine):
    """adapter ad on feature-major input inT; combine(mc, psum_ap) consumes
    the 6 up-projection psum outputs (bias not yet added)."""
    W = ctx.W
    wd, bd = W["w%sd" % ad], W["b%sd" % ad]
    wu = W["w%su" % ad]
    gs = []
    for oc, (ob, osz) in enumerate(((0, P), (P, 64))):
        ps = ctx.pmm.tile([P, 512], F32, tag="mm", name="mmps")
        ps = ps[:, :TAU]
        for k in range(DK):
            nc.tensor.matmul(ps[:osz], wd[:, k, ob:ob + osz], inT[:, k, :],
                             start=(k == 0), stop=(k == DK - 1))
        g = ctx.u.tile([P, TAU], BF, tag="gad%d" % oc)
        # quick_gelu ~ exact gelu to ~1% of value; shares an act-table set
        # with the mlp nonlinearity so the act engine never reloads tables
        # between adapter and mlp stages
        nc.scalar.activation(g[:osz], ps[:osz], AF.Gelu_apprx_sigmoid,
                             bias=bd[:osz, oc:oc + 1])
        gs.append(g)
    for mc in range(DK):
        ps = ctx.pmm.tile([P, 512], F32, tag="mm", name="mmps")
        ps = ps[:, :TAU]
        nc.tensor.matmul(ps[:], wu[:, 0, mc * P:(mc + 1) * P], gs[0][:],
                         start=True, stop=False)
        nc.tensor.matmul(ps[:], wu[:64, 1, mc * P:(mc + 1) * P], gs[1][:64],
                         start=False, stop=True)
        combine(mc, ps)


def emit_attention(ctx, nc, inT, tiles):
    """multi-head attention core: feature-major input inT (post-LN/adapter).
    Returns oT (feature-major, softmax-normalized, pre-proj)."""
    W = ctx.W
    wq = ctx.W["wqkv"]
    # q,k feature-major
    qkT = ctx.qk.tile([P, 2 * DK, TAU], BF, tag="qkT")
    for oc in range(2 * DK):
        ps = ctx.pmm.tile([P, 512], F32, tag="mm", name="mmps")
        ps = ps[:, :TAU]
        for k in range(DK):
            nc.tensor.matmul(ps[:], wq[:, k, oc * P:(oc + 1) * P], inT[:, k, :],
                             start=(k == 0), stop=(k == DK - 1))
        nc.scalar.copy(qkT[:, oc, :], ps[:])
    # v token-major
    vts = []
    for i, (r0, pi, co) in enumerate(tiles):
        vt = ctx.vt.tile([P, D], BF, tag="vtok")
        for nb, nsz in ((0, 512), (512, 256)):
            ps = ctx.pmm.tile([P, 512], F32, tag="mm", name="psv")
            for k in range(DK):
                nc.tensor.matmul(ps[:pi, :nsz], inT[:, k, co:co + pi],
                                 wq[:, k, 2 * D + nb:2 * D + nb + nsz],
                                 start=(k == 0), stop=(k == DK - 1))
            nc.any.tensor_copy(vt[:pi, nb:nb + nsz], ps[:pi, :nsz])
        vts.append(vt)
    oT = ctx.oT.tile([P, DK, TAU], BF, tag="oT")
    kts = ((0, P), (P, NSEQ - P))
    for j in range(2):  # seq in pair
        c0 = j * NSEQ
        for h in range(H):
            qof = 64 * (h % 2)
            qch, kch = h // 2, DK + h // 2
            q = qkT[qof:qof + 64, qch, c0:c0 + NSEQ]
            sT = ctx.psT.tile([P, 2, 256], F32, tag="sT", name="sT")
            sT = sT[:, :, :NSEQ]
            for kt, (kb, kp) in enumerate(kts):
                nc.tensor.matmul(sT[:kp, kt, :],
                                 qkT[qof:qof + 64, kch, c0 + kb:c0 + kb + kp],
                                 q, start=True, stop=True)
            ae = ctx.ae.tile([P, 2, NSEQ], BF, tag="ae")
            for kt, (kb, kp) in enumerate(kts):
                nc.scalar.activation(ae[:kp, kt, :], sT[:kp, kt, :], AF.Exp)
            sm = ctx.prb.tile([P, 512], F32, tag="prb", name="sm")
            sm = sm[:, :NSEQ]
            for kt, (kb, kp) in enumerate(kts):
                nc.tensor.matmul(sm[:1, :], W["ones"][:kp, 0:1], ae[:kp, kt, :],
                                 start=(kt == 0), stop=(kt == 1))
            r = ctx.rr.tile([1, NSEQ], F32, tag="r")
            nc.vector.reciprocal(r[:1], sm[:1, :])
            rb = ctx.rr.tile([1, NSEQ], BF, tag="rb")
            nc.vector.tensor_copy(rb[:1], r[:1])
            pb = ctx.prb.tile([P, 512], F32, tag="prb", name="pb")
            pb = pb[:, :NSEQ]
            nc.tensor.matmul(pb[:], W["ones"][0:1, :P], rb[:1], start=True, stop=True)
            rbs = ctx.rbs.tile([P, NSEQ], F32, tag="rbs")
            nc.vector.tensor_copy(rbs[:], pb[:])
            po = ctx.po.tile([P, 512], F32, tag="po", name="po")
            po = po[:, :NSEQ]
            for kt, (kb, kp) in enumerate(kts):
                nc.tensor.matmul(po[qof:qof + 64, :], vts[2 * j + kt][:kp, h * HD:(h + 1) * HD],
                                 ae[:kp, kt, :], start=(kt == 0), stop=(kt == 1))
            nc.vector.tensor_tensor(oT[qof:qof + 64, qch, c0:c0 + NSEQ],
                                    po[qof:qof + 64, :], rbs[qof:qof + 64, :],
                                    op=OP.mult)
    return oT


def emit_matmul_fm(ctx, nc, wname, kn, inT, combine):
    """dense feature-major matmul: out[:, mc, :] for mc in range(6)."""
    w = ctx.W[wname]
    for mc in range(DK):
        ps = ctx.pmm.tile([P, 512], F32, tag="mm", name="mmps")
        ps = ps[:, :TAU]
        for k in range(kn):
            nc.tensor.matmul(ps[:], w[:, k, mc * P:(mc + 1) * P], inT[:, k, :],
                             start=(k == 0), stop=(k == kn - 1))
        combine(mc, ps)


def emit_fc2(ctx, nc, d, g2, combine):
    for mc in range(DK):
        wt = ctx.wf2.tile([P, HK * P], BF, tag="wf2")
        nc.sync.dma_start(wt[:], d["wfc2"][mc])
        ps = ctx.pmm.tile([P, 512], F32, tag="mm", name="mmps")
        ps = ps[:, :TAU]
        for k in range(HK):
            nc.tensor.matmul(ps[:], wt[:, k * P:(k + 1) * P], g2[:, k, :],
                             start=(k == 0), stop=(k == HK - 1))
        combine(mc, ps)


def emit_delta_add(ctx, nc, deltaT, xts, tiles):
    """transpose feature-major delta and accumulate into token-major xts."""
    W = ctx.W
    for i, (r0, pi, co) in enumerate(tiles):
        for j in range(DK):
            tp = ctx.ptp.tile([P, 1024], BF, tag="tp", name="tp")
            tp = tp[:, :P]
            nc.tensor.transpose(tp[:pi, :P], deltaT[:, j, co:co + pi],
                                W["ident"][:, :])
            nc.vector.tensor_tensor(xts[i][:pi, j * P:(j + 1) * P],
                                    xts[i][:pi, j * P:(j + 1) * P],
                                    tp[:pi, :P], op=OP.add)


def emit_pair_gen(ctx, nc, d, branch, rowbase):
    W = ctx.W
    tiles = PAIR_TILES
    # ---- stage A: load + LN1
    xts = []
    for (r0, pi, co) in tiles:
        xt = ctx.xres.tile([P, D], F32, tag="xres")
        nc.sync.dma_start(xt[:pi], d["x"][bass.ds(rowbase + r0, pi), :])
        xts.append(xt)
    xnT = emit_ln(ctx, nc, xts, tiles, "g1", "b1")
    yield

    # ---- branch-specific pre-attention
    if branch == "T":
        aT = ctx.fmB.tile([P, DK, TAU], BF, tag="fmB")

        def tab_comb(mc, ps):
            nc.scalar.activation(aT[:, mc, :], ps[:], AF.Identity,
                                 bias=W["btabu"][:, mc:mc + 1])
        emit_adapter(ctx, nc, "tab", xnT, tab_comb)
        attn_in = aT
        saT = None
    else:
        saT = ctx.sa.tile([P, DK, TAU], BF, tag="saT")

        def sa_comb(mc, ps):
            nc.scalar.activation(saT[:, mc, :], ps[:], AF.Identity,
                                 bias=W["bsau"][:, mc:mc + 1])
        emit_adapter(ctx, nc, "sa", xnT, sa_comb)
        attn_in = xnT
    yield

    # ---- attention
    oT = emit_attention(ctx, nc, attn_in, tiles)
    yield

    # ---- proj (+ branch combine) -> delta1
    delta1 = ctx.fmC.tile([P, DK, TAU], BF, tag="fmC")
    if branch == "T":
        attnT = ctx.fmB.tile([P, DK, TAU], BF, tag="fmB")

        def proj_comb(mc, ps):
            nc.scalar.activation(attnT[:, mc, :], ps[:], AF.Identity,
                                 bias=W["bproj"][:, mc:mc + 1])
        emit_matmul_fm(ctx, nc, "wproj", DK, oT, proj_comb)

        def ta_comb(mc, ps):
            nc.scalar.activation(delta1[:, mc, :], ps[:], AF.Identity,
                                 bias=W["btau"][:, mc:mc + 1])
        emit_adapter(ctx, nc, "ta", attnT, ta_comb)
    else:
        def proj_comb_s(mc, ps):
            nc.vector.scalar_tensor_tensor(delta1[:, mc, :], ps[:],
                                           W["bproj"][:, mc:mc + 1],
                                           saT[:, mc, :],
                                           op0=OP.add, op1=OP.add)
        emit_matmul_fm(ctx, nc, "wproj", DK, oT, proj_comb_s)

    # ---- first residual: x2 = x + delta1 (in-place on xts)
    emit_delta_add(ctx, nc, delta1, xts, tiles)
    yield

    # ---- LN2
    xn2T = emit_ln(ctx, nc, xts, tiles, "g2", "b2")
    yield

    # ---- MLP (+ sm adapter for spatial)
    if branch == "S":
        smT = ctx.sa.tile([P, DK, TAU], BF, tag="saT")

        def sm_comb(mc, ps):
            nc.scalar.activation(smT[:, mc, :], ps[:], AF.Identity,
                                 bias=W["bsmu"][:, mc:mc + 1])
        emit_adapter(ctx, nc, "sm", xn2T, sm_comb)

    g2 = ctx.g2.tile([P, HK, TAU], BF, tag="g2")
    for oc in range(HK):
        wt = ctx.wf1.tile([P, DK * P], BF, tag="wf1")
        nc.sync.dma_start(wt[:], d["wfc1"][oc])
        ps = ctx.pmm.tile([P, 512], F32, tag="mm", name="mmps")
        ps = ps[:, :TAU]
        for k in range(DK):
            nc.tensor.matmul(ps[:], wt[:, k * P:(k + 1) * P],
                             xn2T[:, k, :], start=(k == 0), stop=(k == DK - 1))
        nc.scalar.activation(g2[:, oc, :], ps[:], AF.Gelu_apprx_sigmoid,
                             bias=W["bfc1s"][:, oc:oc + 1])
    # NOTE: no yield between fc1 and fc2 — g2 is single-buffered, so the
    # other stream's fc1 must not be emitted between our fc1 and fc2 (the
    # write-after-read through the in-order PE queue would deadlock).

    delta2 = ctx.fmC.tile([P, DK, TAU], BF, tag="fmC")
    if branch == "T":
        mlpT = ctx.fmB.tile([P, DK, TAU], BF, tag="fmB")

        def fc2_comb(mc, ps):
            nc.scalar.activation(mlpT[:, mc, :], ps[:], AF.Identity,
                                 bias=W["bfc2"][:, mc:mc + 1])
        emit_fc2(ctx, nc, d, g2, fc2_comb)

        def tm_comb(mc, ps):
            nc.scalar.activation(delta2[:, mc, :], ps[:], AF.Identity,
                                 bias=W["btmu"][:, mc:mc + 1])
        emit_adapter(ctx, nc, "tm", mlpT, tm_comb)
    else:
        def fc2_comb_s(mc, ps):
            nc.vector.scalar_tensor_tensor(delta2[:, mc, :], ps[:],
                                           W["bfc2"][:, mc:mc + 1],
                                           smT[:, mc, :], op0=OP.add, op1=OP.add)
        emit_fc2(ctx, nc, d, g2, fc2_comb_s)

    # ---- second residual + store
    emit_delta_add(ctx, nc, delta2, xts, tiles)
    for i, (r0, pi, co) in enumerate(tiles):
        nc.sync.dma_start(d["y"][bass.ds(rowbase + r0, pi), :], xts[i][:pi, :])


def build_program(npairs=4, loop=True, reps=1):
    import contextlib
    nc = bacc.Bacc("TRN2", target_bir_lowering=False, debug=False,
                   num_devices=NCORES)
    d = {}
    d["x"] = nc.dram_tensor("x", [ROWS, D], F32, kind="ExternalInput").ap()
    for name, shape, dt in WEIGHT_SPECS + STREAMED_SPECS:
        d[name] = nc.dram_tensor(name, shape, dt, kind="ExternalInput").ap()
    d["y"] = nc.dram_tensor("y", [ROWS, D], F32, kind="ExternalOutput").ap()

    with tile.TileContext(nc) as tc:
        with contextlib.ExitStack() as es:
            ctx = Ctx()
            make_pools(ctx, tc, es)
            load_weights(ctx, nc, d)

            def body_pairgroup(i):
                # interleave the two independent pair pipelines stage-by-stage
                # so each stream's serial dependency chains are filled with the
                # other stream's engine work
                gens = [emit_pair_gen(ctx, nc, d, "T", i),
                        emit_pair_gen(ctx, nc, d, "S", i + TT * NSEQ)]
                alive = list(gens)
                while alive:
                    for g in list(alive):
                        try:
                            next(g)
                        except StopIteration:
                            alive.remove(g)

            def body_all():
                if loop:
                    with tc.For_i(0, npairs * TAU, TAU, staggered_reset=True) as i:
                        body_pairgroup(i)
                else:
                    for p in range(npairs):
                        body_pairgroup(p * TAU)

            if reps > 1:
                with tc.For_i(0, reps, 1):
                    body_all()
            else:
                body_all()
            if os.environ.get("KERNEL_POOL_REPORT"):
                for nm in dir(ctx):
                    p = getattr(ctx, nm)
                    if isinstance(p, tile.TilePool):
                        print("pool %-10s %8.2f KB/part  space=%s"
                              % (nm, p.current_size() / (1024 * P), p.space))
    nc.compile()
    return nc


# ----------------------------------------------------------------------------
# harness entry point
# ----------------------------------------------------------------------------

_CACHED = {}


def kernel(**inputs):
    if "nc" not in _CACHED:
        _CACHED["nc"] = build_program()
    nc = _CACHED["nc"]
    w = preprocess_weights(inputs)
    x = np.asarray(inputs["x"], np.float32)  # [128, 197, 768]
    in_maps = []
    for c in range(NCORES):
        m = dict(w)
        m["x"] = np.ascontiguousarray(
            x[c * T:(c + 1) * T].reshape(ROWS, D))
        in_maps.append(m)
    res = run_bass_kernel_spmd(nc, in_maps, core_ids=list(range(NCORES)))
    out = np.stack([r["y"].reshape(T, NSEQ, D) for r in res.results])
    return out.reshape(NCORES * T, NSEQ, D)



# revision 48
# speedup vs baseline: 1.0608x; 1.0608x over previous
"""Trainium2 Bass kernel for the temporal/spatial adapter transformer block.

Sharding: data-parallel over the video batch B=8 -> 1 video (16 frames) per
NeuronCore; all weights replicated. No collectives.

Per-core layout strategy:
  - token-major fp32 residual stream (LayerNorm stats via bn_stats,
    per-partition mean/rstd scalars),
  - feature-major bf16 compute stream for all matmul chains,
  - PE transposes only on bf16 tensors (1 cycle/row),
  - attention computed with transposed scores (S^T = k^T . q) so softmax
    normalization reduces over the partition dim via ones-matmuls; the
    1/sum normalization is deferred onto o via a PE broadcast.
"""

import os
import sys

import numpy as np
import ml_dtypes

try:
    import concourse.bass  # noqa: F401
except ImportError:  # concourse ships with the container, not on sys.path
    for p in ("/opt/trn_rl_repo", "/root/.axon_site/_ro/trn_rl_repo"):
        if p not in sys.path:
            sys.path.insert(0, p)

import concourse.bass as bass
import concourse.mybir as mybir
import concourse.tile as tile
from concourse import bacc
from concourse.bass_utils import run_bass_kernel_spmd

BF = mybir.dt.bfloat16
F32 = mybir.dt.float32
AF = mybir.ActivationFunctionType
OP = mybir.AluOpType

P = 128
NSEQ = 197          # tokens per frame/sequence
D = 768
DK = D // P         # 6
H = 12
HD = 64
BOT = 192
HID = 4 * D         # 3072
HK = HID // P       # 24
EPS = 1e-5
T = 16              # frames per video
TT = 8              # temporal frames
NCORES = 8
TAU = 2 * NSEQ      # tokens per pair = 394
ROWS = T * NSEQ     # 3152 rows per core

GELU_C = 0.044715
GELU_S = 0.7978845608028654  # sqrt(2/pi)
QK_SCALE = HD ** -0.5

ADAPTERS = ("tab", "sa", "ta", "sm", "tm")

bf16 = ml_dtypes.bfloat16


# ----------------------------------------------------------------------------
# host-side weight preprocessing (shared by all cores)
# ----------------------------------------------------------------------------

def preprocess_weights(inp):
    """Build the per-core constant input arrays (already in SBUF layout).

    The layernorm affine transforms are folded into the consumers so the
    on-chip LN is a pure normalize:
      (g*xhat + b) @ W.T + c  ==  xhat @ (W*g).T + (c + W@b)
    qkv has no bias slot; the q/k contributions go into per-chunk biases
    applied in the psum->sbuf copies, and the v contribution is constant
    across keys after softmax (rows sum to 1) so it folds into proj_b.
    """
    w = {}

    def fm(mat):  # [out, in] -> lhsT layout [128, in//128, out]
        o, i = mat.shape
        return np.ascontiguousarray(
            mat.T.reshape(i // P, P, o).transpose(1, 0, 2).astype(np.float32)
        ).astype(bf16)

    g1 = np.asarray(inp["n1_g"], np.float64)
    be1 = np.asarray(inp["n1_b"], np.float64)
    g2v = np.asarray(inp["n2_g"], np.float64)
    be2 = np.asarray(inp["n2_b"], np.float64)

    qkv = np.asarray(inp["qkv_w"], np.float64)
    bqkv = qkv @ be1
    qkvs = qkv * g1[None, :]
    qkvs[:D] *= QK_SCALE  # fold attention scale into q
    bqkv[:D] *= QK_SCALE
    a = fm(qkvs)                                           # [128, 6, 2304]
    # q/k weights streamed per output chunk; v block stays resident
    w["wqks"] = np.ascontiguousarray(
        a[:, :, :2 * D].reshape(P, DK, 2 * DK, P)
        .transpose(2, 0, 1, 3).reshape(2 * DK, P, DK * P)
        .reshape(DK, 2, P, DK * P).transpose(0, 2, 1, 3)
        .reshape(DK, P, 2 * DK * P))
    w["wqv"] = np.ascontiguousarray(a[:, :, 2 * D:])       # [128, 6, 768]
    w["bqk"] = np.ascontiguousarray(
        bqkv[:2 * D].reshape(2 * DK, P).T).astype(np.float32)  # [128, 12]

    proj = np.asarray(inp["proj_w"], np.float64)
    projb = np.asarray(inp["proj_b"], np.float64) + proj @ bqkv[2 * D:]
    w["wproj"] = fm(proj)                                   # [128, 6, 768]
    w["bproj"] = projb.reshape(DK, P).T.astype(np.float32).copy()

    fc1 = np.asarray(inp["fc1_w"], np.float64)
    a = fm(fc1 * g2v[None, :])                              # [128, 6, 3072]
    # streamed as 12 double-chunks [128, 2*768] to halve the DMA count
    w["wfc1"] = np.ascontiguousarray(
        a.reshape(P, DK, HK, P).transpose(2, 0, 1, 3).reshape(HK, P, DK * P)
        .reshape(HK // 2, 2, P, DK * P).transpose(0, 2, 1, 3)
        .reshape(HK // 2, P, 2 * DK * P))
    b1 = np.asarray(inp["fc1_b"], np.float64) + fc1 @ be2
    w["bfc1s"] = b1.reshape(HK, P).T.astype(np.float32).copy()
    a = fm(np.asarray(inp["fc2_w"], np.float64))            # [128, 24, 768]
    w["wfc2"] = np.ascontiguousarray(
        a.reshape(P, HK, DK, P).transpose(2, 0, 1, 3).reshape(DK, P, HK * P))
    w["bfc2"] = np.asarray(inp["fc2_b"], np.float32).reshape(DK, P).T.copy()

    folds = {"tab": (g1, be1), "sa": (g1, be1), "sm": (g2v, be2),
             "ta": None, "tm": None}
    for ad in ADAPTERS:
        dw = np.asarray(inp[ad + "_dw"], np.float64)        # [192, 768]
        db = np.asarray(inp[ad + "_db"], np.float64)        # [192]
        uw = np.asarray(inp[ad + "_uw"], np.float32)        # [768, 192]
        ub = np.asarray(inp[ad + "_ub"], np.float32)        # [768]
        if folds[ad] is not None:
            gv, bev = folds[ad]
            db = db + dw @ bev
            dw = dw * gv[None, :]
        w["w%sd" % ad] = fm(dw)                             # [128, 6, 192]
        bd = np.zeros((P, 2), np.float32)
        bd[:, 0] = db[:P]
        bd[:64, 1] = db[P:]
        w["b%sd" % ad] = bd
        # up: lhsT [192, 768] -> [128, 2, 768], chunk1 rows 64:128 zero
        up = np.zeros((2 * P, D), np.float32)
        up[:BOT] = uw.T
        w["w%su" % ad] = up.reshape(2, P, D).transpose(1, 0, 2).astype(bf16)
        w["b%su" % ad] = ub.reshape(DK, P).T.copy()

    w["ident"] = np.eye(P, dtype=bf16)
    w["ones"] = np.ones((P, P), dtype=bf16)
    w["epsc"] = np.full((P, 1), EPS, np.float32)
    return w


STREAMED_SPECS = [
    ("wfc1", [HK // 2, P, 2 * DK * P], BF),
    ("wfc2", [DK, P, HK * P], BF),
    ("wqks", [DK, P, 2 * DK * P], BF),
]

WEIGHT_SPECS = [
    ("wqv", [P, DK, D], BF), ("bqk", [P, 2 * DK], F32),
    ("wproj", [P, DK, D], BF), ("bproj", [P, DK], F32),
    ("bfc1s", [P, HK], F32),
    ("bfc2", [P, DK], F32),
    ("ident", [P, P], BF), ("ones", [P, P], BF), ("epsc", [P, 1], F32),
] + [
    it for ad in ADAPTERS for it in [
        ("w%sd" % ad, [P, DK, BOT], BF),
        ("b%sd" % ad, [P, 2], F32),
        ("w%su" % ad, [P, 2, D], BF),
        ("b%su" % ad, [P, DK], F32),
    ]
]


# ----------------------------------------------------------------------------
# program emission
# ----------------------------------------------------------------------------

# token tiles of a pair: (row_offset_within_pair, nrows, fm_col_offset)
PAIR_TILES = [(0, P, 0), (P, NSEQ - P, P),
              (NSEQ, P, NSEQ), (NSEQ + P, NSEQ - P, NSEQ + P)]


class Ctx:
    pass


def make_pools(ctx, tc, es):
    def pool(name, bufs):
        return es.enter_context(tc.tile_pool(name=name, bufs=bufs))

    def ppool(name, bufs):
        return es.enter_context(tc.tile_pool(name=name, bufs=bufs, space="PSUM"))

    ctx.weights = pool("weights", 1)
    ctx.xres = pool("xres", 8)       # token-major f32 residual stream
    ctx.small = pool("small", 4)     # bn stats etc.
    ctx.xn = pool("xn", 2)           # token-major bf16 LN output
    ctx.fmA = pool("fmA", 2)         # xnT / xn2T
    ctx.fmB = pool("fmB", 2)         # tab-out / attnT / mlpT (matmul inputs)
    ctx.fmC = pool("fmC", 2)         # delta tiles
    ctx.qk = pool("qk", 2)           # q,k feature-major
    ctx.vt = pool("vt", 4)           # v token-major
    ctx.oT = pool("oT", 2)
    ctx.sa = pool("sa", 2)           # saT / smT
    ctx.ae = pool("ae", 3)           # exp'd scores bf16
    ctx.smb = pool("smb", 2)         # softmax sums bf16 [1, 453]
    ctx.rbs = pool("rbs", 2)         # per-query 1/sum, broadcast [128, 197]
    ctx.g2 = pool("g2", 2)           # mlp gelu output (one per stream)
    ctx.qkw = pool("qkw", 2)         # streamed q/k weight double-chunks
    ctx.wf1 = pool("wf1", 2)         # streamed fc1 weight double-chunks
    ctx.wf2 = pool("wf2", 2)         # streamed fc2 weight tiles (half chunks)
    ctx.u = pool("u", 2)             # adapter gelu outputs bf16

    ctx.pmm = ppool("pmm", 2)        # dense matmul outputs [128, TAU]
    ctx.pp = ppool("pp", 2)          # transposes [128, 8, 128]bf / po [128,512]f32
    ctx.psb = ppool("psb", 2)        # scores sT
    ctx.prb = ppool("prb", 2)        # softmax sums + recip broadcasts


def load_weights(ctx, nc, d):
    ctx.W = {}
    for name, shape, dt in WEIGHT_SPECS:
        t = ctx.weights.tile(shape, dt, tag=name)
        nc.sync.dma_start(t[:], d[name][:])
        ctx.W[name] = t


def emit_ln(ctx, nc, xts, tiles):
    """token-major pure-normalize LN on xts (f32) -> feature-major bf16
    [128, DK, TAU] (the affine transform is folded into consumer weights)."""
    W = ctx.W
    # batched stats: one sqrt + one reciprocal for all 4 token tiles
    mv4 = ctx.small.tile([P, 4, 2], F32, tag="bnmv4")
    for i, (r0, pi, co) in enumerate(tiles):
        xt = xts[i]
        st = ctx.small.tile([P, 2, 6], F32, tag="bnst")
        xf = xt[:pi].rearrange("p a b -> p (a b)")
        nc.vector.bn_stats(st[:pi, 0, :], xf[:, 0:D // 2])
        nc.vector.bn_stats(st[:pi, 1, :], xf[:, D // 2:D])
        nc.vector.bn_aggr(mv4[:pi, i, :], st[:pi])
    sd4 = ctx.small.tile([P, 4], F32, tag="sd4")
    nc.scalar.activation(sd4[:], mv4[:, :, 1], AF.Sqrt, bias=W["epsc"][:, 0:1])
    rstd4 = ctx.small.tile([P, 4], F32, tag="rstd4")
    nc.vector.reciprocal(rstd4[:], sd4[:])
    xnT = ctx.fmA.tile([P, DK, TAU], BF, tag="xnT")
    for i, (r0, pi, co) in enumerate(tiles):
        xn = ctx.xn.tile([P, DK, P], BF, tag="xn")
        nc.vector.tensor_scalar(xn[:pi], xts[i][:pi], mv4[:pi, i, 0:1],
                                rstd4[:pi, i:i + 1],
                                op0=OP.subtract, op1=OP.mult)
        tp = ctx.pp.tile([P, 8, P], BF, tag="pp", name="tpln")
        for j in range(DK):
            nc.tensor.transpose(tp[:, j, :pi], xn[:pi, j, :],
                                W["ident"][:pi, :pi])
        nc.vector.tensor_copy(xnT[:, :, co:co + pi], tp[:, 0:DK, 0:pi])
    return xnT


def emit_adapter(ctx, nc, ad, inT, combine):
    """adapter ad on feature-major input inT; combine(mc, psum_ap) consumes
    the 6 up-projection psum outputs (bias not yet added)."""
    W = ctx.W
    wd, bd = W["w%sd" % ad], W["b%sd" % ad]
    wu = W["w%su" % ad]
    gs = []
    for oc, (ob, osz) in enumerate(((0, P), (P, 64))):
        ps = ctx.pmm.tile([P, 512], F32, tag="mm", name="mmps")
        ps = ps[:, :TAU]
        for k in range(DK):
            nc.tensor.matmul(ps[:osz], wd[:, k, ob:ob + osz], inT[:, k, :],
                             start=(k == 0), stop=(k == DK - 1))
        g = ctx.u.tile([P, TAU], BF, tag="gad%d" % oc)
        # quick_gelu ~ exact gelu to ~1% of value; shares an act-table set
        # with the mlp nonlinearity so the act engine never reloads tables
        # between adapter and mlp stages
        nc.scalar.activation(g[:osz], ps[:osz], AF.Gelu_apprx_sigmoid,
                             bias=bd[:osz, oc:oc + 1])
        gs.append(g)
    for mc in range(DK):
        ps = ctx.pmm.tile([P, 512], F32, tag="mm", name="mmps")
        ps = ps[:, :TAU]
        nc.tensor.matmul(ps[:], wu[:, 0, mc * P:(mc + 1) * P], gs[0][:],
                         start=True, stop=False)
        nc.tensor.matmul(ps[:], wu[:64, 1, mc * P:(mc + 1) * P], gs[1][:64],
                         start=False, stop=True)
        combine(mc, ps)


def emit_attention(ctx, nc, d, inT, tiles):
    """multi-head attention core: feature-major input inT (post-LN/adapter).
    Returns oT (feature-major, softmax-normalized, pre-proj).

    Processed per head-pair (the two heads sharing a 128-row feature chunk),
    software-pipelined: phase 1 (scores/exp/sums for unit u) is emitted before
    phase 2 (broadcast/o/normalize of unit u-1) so the PE never waits on the
    softmax-sum round trip."""
    W = ctx.W
    # v first (resident weights, no DMA dependency) so the q/k weight
    # stream gets a head start
    wv = W["wqv"]
    vts = []
    for i, (r0, pi, co) in enumerate(tiles):
        vt = ctx.vt.tile([P, D], BF, tag="vtok")
        for nb, nsz in ((0, 512), (512, 256)):
            ps = ctx.pmm.tile([P, 512], F32, tag="mm", name="psv")
            for k in range(DK):
                nc.tensor.matmul(ps[:pi, :nsz], inT[:, k, co:co + pi],
                                 wv[:, k, nb:nb + nsz],
                                 start=(k == 0), stop=(k == DK - 1))
            nc.any.tensor_copy(vt[:pi, nb:nb + nsz], ps[:pi, :nsz])
        vts.append(vt)
    # q,k feature-major (bias from folded LN shift applied in the copy);
    # q/k weights streamed as double chunks, v block is resident
    qkT = ctx.qk.tile([P, 2 * DK, TAU], BF, tag="qkT")
    for ocp in range(DK):
        wqk = ctx.qkw.tile([P, 2 * DK * P], BF, tag="qkw")
        nc.sync.dma_start(wqk[:], d["wqks"][ocp])
        for sub in range(2):
            oc = 2 * ocp + sub
            ps = ctx.pmm.tile([P, 512], F32, tag="mm", name="mmps")
            ps = ps[:, :TAU]
            for k in range(DK):
                nc.tensor.matmul(ps[:], wqk[:, (sub * DK + k) * P:
                                             (sub * DK + k + 1) * P],
                                 inT[:, k, :],
                                 start=(k == 0), stop=(k == DK - 1))
            nc.scalar.activation(qkT[:, oc, :], ps[:], AF.Identity,
                                 bias=W["bqk"][:, oc:oc + 1])
    oT = ctx.oT.tile([P, DK, TAU], BF, tag="oT")
    kts = ((0, P), (P, NSEQ - P))

    def phase2(st):
        j, hp, aes, smb = st
        c0 = j * NSEQ
        # broadcast the two sums rows over the head-pair's partition halves
        pb = ctx.prb.tile([P, 512], F32, tag="prb", name="pb")
        nc.tensor.matmul(pb[0:64, :NSEQ], W["ones"][0:1, 0:64],
                         smb[0:1, 0:NSEQ], start=True, stop=True)
        nc.tensor.matmul(pb[64:128, :NSEQ], W["ones"][0:1, 0:64],
                         smb[0:1, 256:256 + NSEQ], start=True, stop=True)
        po = ctx.pp.tile([P, 512], F32, tag="pp", name="po")
        for hi in range(2):
            h = 2 * hp + hi
            ob = 64 * hi
            for kt, (kb, kp) in enumerate(kts):
                nc.tensor.matmul(po[ob:ob + 64, :NSEQ],
                                 vts[2 * j + kt][:kp, h * HD:(h + 1) * HD],
                                 aes[hi][:kp, kt, :], start=(kt == 0),
                                 stop=(kt == 1))
        rbs = ctx.rbs.tile([P, NSEQ], F32, tag="rbs")
        nc.vector.reciprocal(rbs[:], pb[:, :NSEQ])
        nc.vector.tensor_tensor(oT[:, hp, c0:c0 + NSEQ], po[:, :NSEQ],
                                rbs[:], op=OP.mult)

    prev = None
    for j in range(2):  # seq in pair
        c0 = j * NSEQ
        for hp in range(DK):  # head pair = feature chunk
            aes = []
            for hi in range(2):
                h = 2 * hp + hi
                qof = 64 * (h % 2)
                qch, kch = h // 2, DK + h // 2
                q = qkT[qof:qof + 64, qch, c0:c0 + NSEQ]
                sT = ctx.psb.tile([P, 2, 256], F32, tag="psb", name="sT")
                for kt, (kb, kp) in enumerate(kts):
                    nc.tensor.matmul(sT[:kp, kt, :NSEQ],
                                     qkT[qof:qof + 64, kch, c0 + kb:c0 + kb + kp],
                                     q, start=True, stop=True)
                ae = ctx.ae.tile([P, 2, NSEQ], BF, tag="ae")
                # one exp over both key-chunks (rows 69:128 of chunk 1 hold
                # stale finite psum values; they are never read downstream)
                nc.scalar.activation(ae[:, :, :], sT[:, :, :NSEQ], AF.Exp)
                aes.append(ae)
            sm = ctx.prb.tile([P, 512], F32, tag="prb", name="sm")
            for hi in range(2):
                cb = 256 * hi
                for kt, (kb, kp) in enumerate(kts):
                    nc.tensor.matmul(sm[0:1, cb:cb + NSEQ], W["ones"][:kp, 0:1],
                                     aes[hi][:kp, kt, :], start=(kt == 0),
                                     stop=(kt == 1))
            smb = ctx.smb.tile([1, 453], BF, tag="smb")
            nc.scalar.activation(smb[0:1, :], sm[0:1, 0:453], AF.Identity)
            if prev is not None:
                phase2(prev)
            prev = (j, hp, aes, smb)
    phase2(prev)
    return oT


def emit_matmul_fm(ctx, nc, wname, kn, inT, combine):
    """dense feature-major matmul: out[:, mc, :] for mc in range(6)."""
    w = ctx.W[wname]
    for mc in range(DK):
        ps = ctx.pmm.tile([P, 512], F32, tag="mm", name="mmps")
        ps = ps[:, :TAU]
        for k in range(kn):
            nc.tensor.matmul(ps[:], w[:, k, mc * P:(mc + 1) * P], inT[:, k, :],
                             start=(k == 0), stop=(k == kn - 1))
        combine(mc, ps)


def emit_delta_add(ctx, nc, deltaT, xts, tiles):
    """transpose feature-major delta and accumulate into token-major xts;
    all 6 feature chunks of a token tile share one psum bank so the
    accumulate is a single wide vector op."""
    W = ctx.W
    for i, (r0, pi, co) in enumerate(tiles):
        tp = ctx.pp.tile([P, 8, P], BF, tag="pp", name="tpd")
        for j in range(DK):
            nc.tensor.transpose(tp[:pi, j, :], deltaT[:, j, co:co + pi],
                                W["ident"][:, :])
        nc.vector.tensor_tensor(xts[i][:pi], xts[i][:pi],
                                tp[:pi, 0:DK, :], op=OP.add)


def emit_pair_gen(ctx, nc, d, branch, rowbase):
    W = ctx.W
    tiles = PAIR_TILES
    # ---- stage A: load + LN1
    xts = []
    for (r0, pi, co) in tiles:
        xt = ctx.xres.tile([P, DK, P], F32, tag="xres")
        nc.sync.dma_start(xt[:pi], d["x"][bass.ds(rowbase + r0, pi), :, :])
        xts.append(xt)
    xnT = emit_ln(ctx, nc, xts, tiles)
    yield

    # ---- branch-specific pre-attention
    if branch == "T":
        aT = ctx.fmB.tile([P, DK, TAU], BF, tag="fmB")

        def tab_comb(mc, ps):
            nc.scalar.activation(aT[:, mc, :], ps[:], AF.Identity,
                                 bias=W["btabu"][:, mc:mc + 1])
        emit_adapter(ctx, nc, "tab", xnT, tab_comb)
        attn_in = aT
        saT = None
    else:
        saT = ctx.sa.tile([P, DK, TAU], BF, tag="saT")

        def sa_comb(mc, ps):
            nc.scalar.activation(saT[:, mc, :], ps[:], AF.Identity,
                                 bias=W["bsau"][:, mc:mc + 1])
        emit_adapter(ctx, nc, "sa", xnT, sa_comb)
        attn_in = xnT
    yield

    # ---- attention
    oT = emit_attention(ctx, nc, d, attn_in, tiles)
    yield

    # ---- proj (+ branch combine) -> delta1
    delta1 = ctx.fmC.tile([P, DK, TAU], BF, tag="fmC")
    if branch == "T":
        attnT = ctx.fmB.tile([P, DK, TAU], BF, tag="fmB")

        def proj_comb(mc, ps):
            nc.scalar.activation(attnT[:, mc, :], ps[:], AF.Identity,
                                 bias=W["bproj"][:, mc:mc + 1])
        emit_matmul_fm(ctx, nc, "wproj", DK, oT, proj_comb)

        def ta_comb(mc, ps):
            nc.scalar.activation(delta1[:, mc, :], ps[:], AF.Identity,
                                 bias=W["btau"][:, mc:mc + 1])
        emit_adapter(ctx, nc, "ta", attnT, ta_comb)
    else:
        def proj_comb_s(mc, ps):
            nc.vector.scalar_tensor_tensor(delta1[:, mc, :], ps[:],
                                           W["bproj"][:, mc:mc + 1],
                                           saT[:, mc, :],
                                           op0=OP.add, op1=OP.add)
        emit_matmul_fm(ctx, nc, "wproj", DK, oT, proj_comb_s)

    # ---- first residual: x2 = x + delta1 (in-place on xts)
    emit_delta_add(ctx, nc, delta1, xts, tiles)
    yield

    # ---- LN2 (+ sm adapter for spatial); MLP runs fused across streams
    xn2T = emit_ln(ctx, nc, xts, tiles)
    smT = None
    if branch == "S":
        smT = ctx.sa.tile([P, DK, TAU], BF, tag="saT")

        def sm_comb(mc, ps):
            nc.scalar.activation(smT[:, mc, :], ps[:], AF.Identity,
                                 bias=W["bsmu"][:, mc:mc + 1])
        emit_adapter(ctx, nc, "sm", xn2T, sm_comb)
    yield {"branch": branch, "rowbase": rowbase, "xts": xts,
           "xn2T": xn2T, "smT": smT}


def emit_mlp_fused(ctx, nc, d, sts):
    """fc1/fc2 over both streams with each weight chunk streamed once."""
    W = ctx.W
    tiles = PAIR_TILES
    g2s = [ctx.g2.tile([P, HK, TAU], BF, tag="g2", name="g2")
           for _ in sts]
    for ocp in range(HK // 2):
        wt = ctx.wf1.tile([P, 2 * DK * P], BF, tag="wf1")
        nc.scalar.dma_start(wt[:], d["wfc1"][ocp])
        for sub in range(2):
            oc = 2 * ocp + sub
            for st, g2 in zip(sts, g2s):
                ps = ctx.pmm.tile([P, 512], F32, tag="mm", name="mmps")
                ps = ps[:, :TAU]
                for k in range(DK):
                    nc.tensor.matmul(ps[:], wt[:, (sub * DK + k) * P:
                                                (sub * DK + k + 1) * P],
                                     st["xn2T"][:, k, :], start=(k == 0),
                                     stop=(k == DK - 1))
                nc.scalar.activation(g2[:, oc, :], ps[:],
                                     AF.Gelu_apprx_sigmoid,
                                     bias=W["bfc1s"][:, oc:oc + 1])
    mlpTs = {}
    deltas = {}
    for st in sts:
        deltas[st["branch"]] = ctx.fmC.tile([P, DK, TAU], BF,
                                            tag="fmC", name="delta2")
        if st["branch"] == "T":
            mlpTs["T"] = ctx.fmB.tile([P, DK, TAU], BF, tag="fmB",
                                      name="mlpT")
    for mc in range(DK):
        pss = []
        for st in sts:
            psf = ctx.pmm.tile([P, 512], F32, tag="mm", name="mmps")
            pss.append(psf[:, :TAU])
        for half in range(2):
            wt = ctx.wf2.tile([P, HK * P // 2], BF, tag="wf2")
            nc.scalar.dma_start(wt[:], d["wfc2"][mc, :,
                                                 half * HK * P // 2:
                                                 (half + 1) * HK * P // 2])
            for st, g2, ps in zip(sts, g2s, pss):
                for k in range(HK // 2):
                    nc.tensor.matmul(ps[:], wt[:, k * P:(k + 1) * P],
                                     g2[:, HK // 2 * half + k, :],
                                     start=(half == 0 and k == 0),
                                     stop=(half == 1 and k == HK // 2 - 1))
        for st, ps in zip(sts, pss):
            if st["branch"] == "T":
                nc.scalar.activation(mlpTs["T"][:, mc, :], ps[:], AF.Identity,
                                     bias=W["bfc2"][:, mc:mc + 1])
            else:
                nc.vector.scalar_tensor_tensor(deltas["S"][:, mc, :], ps[:],
                                               W["bfc2"][:, mc:mc + 1],
                                               st["smT"][:, mc, :],
                                               op0=OP.add, op1=OP.add)
    for st in sts:
        if st["branch"] == "T":
            def tm_comb(mc, ps):
                nc.scalar.activation(deltas["T"][:, mc, :], ps[:], AF.Identity,
                                     bias=W["btmu"][:, mc:mc + 1])
            emit_adapter(ctx, nc, "tm", mlpTs["T"], tm_comb)
    for st in sts:
        emit_delta_add(ctx, nc, deltas[st["branch"]], st["xts"], tiles)
        for i, (r0, pi, co) in enumerate(tiles):
            nc.sync.dma_start(d["y"][bass.ds(st["rowbase"] + r0, pi), :, :],
                              st["xts"][i][:pi])


def build_program(npairs=4, loop=True, reps=1):
    import contextlib
    nc = bacc.Bacc("TRN2", target_bir_lowering=False, debug=False,
                   num_devices=NCORES)
    d = {}
    d["x"] = nc.dram_tensor("x", [ROWS, DK, P], F32, kind="ExternalInput").ap()
    for name, shape, dt in WEIGHT_SPECS + STREAMED_SPECS:
        d[name] = nc.dram_tensor(name, shape, dt, kind="ExternalInput").ap()
    d["y"] = nc.dram_tensor("y", [ROWS, DK, P], F32, kind="ExternalOutput").ap()

    with tile.TileContext(nc) as tc:
        with contextlib.ExitStack() as es:
            ctx = Ctx()
            make_pools(ctx, tc, es)
            load_weights(ctx, nc, d)

            def body_pairgroup(i):
                # interleave the two independent pair pipelines stage-by-stage
                # so each stream's serial dependency chains are filled with the
                # other stream's engine work; the MLP runs fused across both
                # streams so each fc weight chunk is streamed once per group
                gens = [emit_pair_gen(ctx, nc, d, "T", i),
                        emit_pair_gen(ctx, nc, d, "S", i + TT * NSEQ)]
                alive = list(gens)
                sts = []
                while alive:
                    for g in list(alive):
                        try:
                            st = next(g)
                            if st is not None:
                                sts.append(st)
                        except StopIteration:
                            alive.remove(g)
                emit_mlp_fused(ctx, nc, d, sts)

            def body_all():
                if loop:
                    with tc.For_i(0, npairs * TAU, TAU, staggered_reset=True) as i:
                        body_pairgroup(i)
                else:
                    for p in range(npairs):
                        body_pairgroup(p * TAU)

            if reps > 1:
                with tc.For_i(0, reps, 1):
                    body_all()
            else:
                body_all()
            if os.environ.get("KERNEL_POOL_REPORT"):
                for nm in dir(ctx):
                    p = getattr(ctx, nm)
                    if isinstance(p, tile.TilePool):
                        print("pool %-10s %8.2f KB/part  space=%s"
                              % (nm, p.current_size() / (1024 * P), p.space))
    nc.compile()
    return nc


# ----------------------------------------------------------------------------
# harness entry point
# ----------------------------------------------------------------------------

_CACHED = {}


def kernel(**inputs):
    if "nc" not in _CACHED:
        _CACHED["nc"] = build_program()
    nc = _CACHED["nc"]
    w = preprocess_weights(inputs)
    x = np.asarray(inputs["x"], np.float32)  # [128, 197, 768]
    in_maps = []
    for c in range(NCORES):
        m = dict(w)
        m["x"] = np.ascontiguousarray(
            x[c * T:(c + 1) * T].reshape(ROWS, DK, P))
        in_maps.append(m)
    res = run_bass_kernel_spmd(nc, in_maps, core_ids=list(range(NCORES)))
    out = np.stack([r["y"].reshape(T, NSEQ, D) for r in res.results])
    return out.reshape(NCORES * T, NSEQ, D)



# revision 51
# speedup vs baseline: 1.5047x; 1.4185x over previous
"""Trainium2 Bass kernel for the temporal/spatial adapter transformer block.

Sharding: data-parallel over the video batch B=8 -> 1 video (16 frames) per
NeuronCore; all weights replicated. No collectives.

Per-core layout strategy:
  - token-major fp32 residual stream (LayerNorm stats via bn_stats,
    per-partition mean/rstd scalars),
  - feature-major bf16 compute stream for all matmul chains,
  - PE transposes only on bf16 tensors (1 cycle/row),
  - attention computed with transposed scores (S^T = k^T . q) so softmax
    normalization reduces over the partition dim via ones-matmuls; the
    1/sum normalization is deferred onto o via a PE broadcast.
"""

import os
import sys

import numpy as np
import ml_dtypes

try:
    import concourse.bass  # noqa: F401
except ImportError:  # concourse ships with the container, not on sys.path
    for p in ("/opt/trn_rl_repo", "/root/.axon_site/_ro/trn_rl_repo"):
        if p not in sys.path:
            sys.path.insert(0, p)

import concourse.bass as bass
import concourse.mybir as mybir
import concourse.tile as tile
from concourse import bacc
from concourse.bass_utils import run_bass_kernel_spmd

BF = mybir.dt.bfloat16
F32 = mybir.dt.float32
AF = mybir.ActivationFunctionType
OP = mybir.AluOpType

P = 128
NSEQ = 197          # tokens per frame/sequence
D = 768
DK = D // P         # 6
H = 12
HD = 64
BOT = 192
HID = 4 * D         # 3072
HK = HID // P       # 24
EPS = 1e-5
T = 16              # frames per video
TT = 8              # temporal frames
NCORES = 8
TAU = 2 * NSEQ      # tokens per pair = 394
ROWS = T * NSEQ     # 3152 rows per core

GELU_C = 0.044715
GELU_S = 0.7978845608028654  # sqrt(2/pi)
QK_SCALE = HD ** -0.5

ADAPTERS = ("tab", "sa", "ta", "sm", "tm")

bf16 = ml_dtypes.bfloat16


# ----------------------------------------------------------------------------
# host-side weight preprocessing (shared by all cores)
# ----------------------------------------------------------------------------

def preprocess_weights(inp):
    """Build the per-core constant input arrays (already in SBUF layout).

    The layernorm affine transforms are folded into the consumers so the
    on-chip LN is a pure normalize:
      (g*xhat + b) @ W.T + c  ==  xhat @ (W*g).T + (c + W@b)
    qkv has no bias slot; the q/k contributions go into per-chunk biases
    applied in the psum->sbuf copies, and the v contribution is constant
    across keys after softmax (rows sum to 1) so it folds into proj_b.
    """
    w = {}

    def fm(mat):  # [out, in] -> lhsT layout [128, in//128, out]
        o, i = mat.shape
        return np.ascontiguousarray(
            mat.T.reshape(i // P, P, o).transpose(1, 0, 2).astype(np.float32)
        ).astype(bf16)

    g1 = np.asarray(inp["n1_g"], np.float64)
    be1 = np.asarray(inp["n1_b"], np.float64)
    g2v = np.asarray(inp["n2_g"], np.float64)
    be2 = np.asarray(inp["n2_b"], np.float64)

    qkv = np.asarray(inp["qkv_w"], np.float64)
    bqkv = qkv @ be1
    qkvs = qkv * g1[None, :]
    qkvs[:D] *= QK_SCALE  # fold attention scale into q
    bqkv[:D] *= QK_SCALE
    a = fm(qkvs)                                           # [128, 6, 2304]
    # q/k weights streamed per output chunk; v block stays resident
    w["wqks"] = np.ascontiguousarray(
        a[:, :, :2 * D].reshape(P, DK, 2 * DK, P)
        .transpose(2, 0, 1, 3).reshape(2 * DK, P, DK * P)
        .reshape(DK, 2, P, DK * P).transpose(0, 2, 1, 3)
        .reshape(DK, P, 2 * DK * P))
    w["wqv"] = np.ascontiguousarray(a[:, :, 2 * D:])       # [128, 6, 768]
    w["bqk"] = np.ascontiguousarray(
        bqkv[:2 * D].reshape(2 * DK, P).T).astype(np.float32)  # [128, 12]

    proj = np.asarray(inp["proj_w"], np.float64)
    projb = np.asarray(inp["proj_b"], np.float64) + proj @ bqkv[2 * D:]
    w["wproj"] = fm(proj)                                   # [128, 6, 768]
    w["bproj"] = projb.reshape(DK, P).T.astype(np.float32).copy()

    fc1 = np.asarray(inp["fc1_w"], np.float64)
    a = fm(fc1 * g2v[None, :])                              # [128, 6, 3072]
    # streamed as 12 double-chunks [128, 2*768] to halve the DMA count
    w["wfc1"] = np.ascontiguousarray(
        a.reshape(P, DK, HK, P).transpose(2, 0, 1, 3).reshape(HK, P, DK * P)
        .reshape(HK // 2, 2, P, DK * P).transpose(0, 2, 1, 3)
        .reshape(HK // 2, P, 2 * DK * P))
    b1 = np.asarray(inp["fc1_b"], np.float64) + fc1 @ be2
    w["bfc1s"] = b1.reshape(HK, P).T.astype(np.float32).copy()
    a = fm(np.asarray(inp["fc2_w"], np.float64))            # [128, 24, 768]
    w["wfc2"] = np.ascontiguousarray(
        a.reshape(P, HK, DK, P).transpose(2, 0, 1, 3).reshape(DK, P, HK * P))
    w["bfc2"] = np.asarray(inp["fc2_b"], np.float32).reshape(DK, P).T.copy()

    folds = {"tab": (g1, be1), "sa": (g1, be1), "sm": (g2v, be2),
             "ta": None, "tm": None}
    for ad in ADAPTERS:
        dw = np.asarray(inp[ad + "_dw"], np.float64)        # [192, 768]
        db = np.asarray(inp[ad + "_db"], np.float64)        # [192]
        uw = np.asarray(inp[ad + "_uw"], np.float32)        # [768, 192]
        ub = np.asarray(inp[ad + "_ub"], np.float32)        # [768]
        if folds[ad] is not None:
            gv, bev = folds[ad]
            db = db + dw @ bev
            dw = dw * gv[None, :]
        w["w%sd" % ad] = fm(dw)                             # [128, 6, 192]
        bd = np.zeros((P, 2), np.float32)
        bd[:, 0] = db[:P]
        bd[:64, 1] = db[P:]
        w["b%sd" % ad] = bd
        # up: lhsT [192, 768] -> [128, 2, 768], chunk1 rows 64:128 zero
        up = np.zeros((2 * P, D), np.float32)
        up[:BOT] = uw.T
        w["w%su" % ad] = up.reshape(2, P, D).transpose(1, 0, 2).astype(bf16)
        w["b%su" % ad] = ub.reshape(DK, P).T.copy()

    w["ident"] = np.eye(P, dtype=bf16)
    w["ones"] = np.ones((P, P), dtype=bf16)
    w["epsc"] = np.full((P, 1), EPS, np.float32)
    return w


STREAMED_SPECS = [
    ("wfc1", [HK // 2, P, 2 * DK * P], BF),
    ("wfc2", [DK, P, HK * P], BF),
    ("wqks", [DK, P, 2 * DK * P], BF),
]

WEIGHT_SPECS = [
    ("wqv", [P, DK, D], BF), ("bqk", [P, 2 * DK], F32),
    ("wproj", [P, DK, D], BF), ("bproj", [P, DK], F32),
    ("bfc1s", [P, HK], F32),
    ("bfc2", [P, DK], F32),
    ("ident", [P, P], BF), ("ones", [P, P], BF), ("epsc", [P, 1], F32),
] + [
    it for ad in ADAPTERS for it in [
        ("w%sd" % ad, [P, DK, BOT], BF),
        ("b%sd" % ad, [P, 2], F32),
        ("w%su" % ad, [P, 2, D], BF),
        ("b%su" % ad, [P, DK], F32),
    ]
]


# ----------------------------------------------------------------------------
# program emission
# ----------------------------------------------------------------------------

# token tiles of a pair: (row_offset_within_pair, nrows, fm_col_offset)
PAIR_TILES = [(0, P, 0), (P, NSEQ - P, P),
              (NSEQ, P, NSEQ), (NSEQ + P, NSEQ - P, NSEQ + P)]


class Ctx:
    pass


def make_pools(ctx, tc, es):
    def pool(name, bufs):
        return es.enter_context(tc.tile_pool(name=name, bufs=bufs))

    def ppool(name, bufs):
        return es.enter_context(tc.tile_pool(name=name, bufs=bufs, space="PSUM"))

    ctx.weights = pool("weights", 1)
    ctx.xres = pool("xres", 8)       # token-major f32 residual stream
    ctx.small = pool("small", 4)     # bn stats etc.
    ctx.xn = pool("xn", 2)           # token-major bf16 LN output
    ctx.fmA = pool("fmA", 2)         # xnT / xn2T
    ctx.fmB = pool("fmB", 2)         # tab-out / attnT / mlpT (matmul inputs)
    ctx.fmC = pool("fmC", 2)         # delta tiles
    ctx.qk = pool("qk", 2)           # q,k feature-major
    ctx.vt = pool("vt", 4)           # v token-major
    ctx.oT = pool("oT", 2)
    ctx.sa = pool("sa", 2)           # saT / smT
    ctx.ae = pool("ae", 3)           # exp'd scores bf16
    ctx.smb = pool("smb", 2)         # softmax sums bf16 [1, 453]
    ctx.rbs = pool("rbs", 2)         # per-query 1/sum, broadcast [128, 197]
    ctx.g2 = pool("g2", 2)           # mlp gelu output (one per stream)
    ctx.qkw = pool("qkw", 2)         # streamed q/k weight double-chunks
    ctx.wf1 = pool("wf1", 2)         # streamed fc1 weight double-chunks
    ctx.wf2 = pool("wf2", 2)         # streamed fc2 weight tiles (half chunks)
    ctx.u = pool("u", 2)             # adapter gelu outputs bf16

    ctx.pmm = ppool("pmm", 2)        # dense matmul outputs [128, TAU]
    ctx.pp = ppool("pp", 2)          # transposes [128, 8, 128]bf / po [128,512]f32
    ctx.psb = ppool("psb", 2)        # scores sT
    ctx.prb = ppool("prb", 2)        # softmax sums + recip broadcasts


def load_weights(ctx, nc, d):
    ctx.W = {}
    for name, shape, dt in WEIGHT_SPECS:
        t = ctx.weights.tile(shape, dt, tag=name)
        nc.sync.dma_start(t[:], d[name][:])
        ctx.W[name] = t


def emit_ln(ctx, nc, xts, tiles):
    """token-major pure-normalize LN on xts (f32) -> feature-major bf16
    [128, DK, TAU] (the affine transform is folded into consumer weights)."""
    W = ctx.W
    # batched stats: one sqrt + one reciprocal for all 4 token tiles
    mv4 = ctx.small.tile([P, 4, 2], F32, tag="bnmv4")
    for i, (r0, pi, co) in enumerate(tiles):
        xt = xts[i]
        st = ctx.small.tile([P, 2, 6], F32, tag="bnst")
        xf = xt[:pi].rearrange("p a b -> p (a b)")
        nc.vector.bn_stats(st[:pi, 0, :], xf[:, 0:D // 2])
        nc.vector.bn_stats(st[:pi, 1, :], xf[:, D // 2:D])
        nc.vector.bn_aggr(mv4[:pi, i, :], st[:pi])
    sd4 = ctx.small.tile([P, 4], F32, tag="sd4")
    nc.scalar.activation(sd4[:], mv4[:, :, 1], AF.Sqrt, bias=W["epsc"][:, 0:1])
    rstd4 = ctx.small.tile([P, 4], F32, tag="rstd4")
    nc.vector.reciprocal(rstd4[:], sd4[:])
    xnT = ctx.fmA.tile([P, DK, TAU], BF, tag="xnT")
    for i, (r0, pi, co) in enumerate(tiles):
        xn = ctx.xn.tile([P, DK, P], BF, tag="xn")
        nc.vector.tensor_scalar(xn[:pi], xts[i][:pi], mv4[:pi, i, 0:1],
                                rstd4[:pi, i:i + 1],
                                op0=OP.subtract, op1=OP.mult)
        tp = ctx.pp.tile([P, 8, P], BF, tag="pp", name="tpln")
        for j in range(DK):
            nc.tensor.transpose(tp[:, j, :pi], xn[:pi, j, :],
                                W["ident"][:pi, :pi])
        nc.vector.tensor_copy(xnT[:, :, co:co + pi], tp[:, 0:DK, 0:pi])
    return xnT


def emit_adapter(ctx, nc, ad, inT, combine):
    """adapter ad on feature-major input inT; combine(mc, psum_ap) consumes
    the 6 up-projection psum outputs (bias not yet added)."""
    W = ctx.W
    wd, bd = W["w%sd" % ad], W["b%sd" % ad]
    wu = W["w%su" % ad]
    gs = []
    for oc, (ob, osz) in enumerate(((0, P), (P, 64))):
        ps = ctx.pmm.tile([P, 512], F32, tag="mm", name="mmps")
        ps = ps[:, :TAU]
        for k in range(DK):
            nc.tensor.matmul(ps[:osz], wd[:, k, ob:ob + osz], inT[:, k, :],
                             start=(k == 0), stop=(k == DK - 1))
        g = ctx.u.tile([P, TAU], BF, tag="gad%d" % oc)
        # quick_gelu ~ exact gelu to ~1% of value; shares an act-table set
        # with the mlp nonlinearity so the act engine never reloads tables
        # between adapter and mlp stages
        nc.scalar.activation(g[:osz], ps[:osz], AF.Gelu_apprx_sigmoid,
                             bias=bd[:osz, oc:oc + 1])
        gs.append(g)
    for mc in range(DK):
        ps = ctx.pmm.tile([P, 512], F32, tag="mm", name="mmps")
        ps = ps[:, :TAU]
        nc.tensor.matmul(ps[:], wu[:, 0, mc * P:(mc + 1) * P], gs[0][:],
                         start=True, stop=False)
        nc.tensor.matmul(ps[:], wu[:64, 1, mc * P:(mc + 1) * P], gs[1][:64],
                         start=False, stop=True)
        combine(mc, ps)


def emit_attention(ctx, nc, d, inT, tiles):
    """multi-head attention core: feature-major input inT (post-LN/adapter).
    Returns oT (feature-major, softmax-normalized, pre-proj).

    Processed per head-pair (the two heads sharing a 128-row feature chunk),
    software-pipelined: phase 1 (scores/exp/sums for unit u) is emitted before
    phase 2 (broadcast/o/normalize of unit u-1) so the PE never waits on the
    softmax-sum round trip."""
    W = ctx.W
    # v first (resident weights, no DMA dependency) so the q/k weight
    # stream gets a head start
    wv = W["wqv"]
    vts = []
    for i, (r0, pi, co) in enumerate(tiles):
        vt = ctx.vt.tile([P, D], BF, tag="vtok")
        for nb, nsz in ((0, 512), (512, 256)):
            ps = ctx.pmm.tile([P, 512], F32, tag="mm", name="psv")
            for k in range(DK):
                nc.tensor.matmul(ps[:pi, :nsz], inT[:, k, co:co + pi],
                                 wv[:, k, nb:nb + nsz],
                                 start=(k == 0), stop=(k == DK - 1))
            nc.any.tensor_copy(vt[:pi, nb:nb + nsz], ps[:pi, :nsz])
        vts.append(vt)
    # q,k feature-major (bias from folded LN shift applied in the copy);
    # q/k weights streamed as double chunks, v block is resident
    qkT = ctx.qk.tile([P, 2 * DK, TAU], BF, tag="qkT")
    for ocp in range(DK):
        wqk = ctx.qkw.tile([P, 2 * DK * P], BF, tag="qkw")
        nc.sync.dma_start(wqk[:], d["wqks"][ocp])
        for sub in range(2):
            oc = 2 * ocp + sub
            ps = ctx.pmm.tile([P, 512], F32, tag="mm", name="mmps")
            ps = ps[:, :TAU]
            for k in range(DK):
                nc.tensor.matmul(ps[:], wqk[:, (sub * DK + k) * P:
                                             (sub * DK + k + 1) * P],
                                 inT[:, k, :],
                                 start=(k == 0), stop=(k == DK - 1))
            nc.scalar.activation(qkT[:, oc, :], ps[:], AF.Identity,
                                 bias=W["bqk"][:, oc:oc + 1])
    oT = ctx.oT.tile([P, DK, TAU], BF, tag="oT")
    kts = ((0, P), (P, NSEQ - P))

    def phase2(st):
        j, hp, aes, smb = st
        c0 = j * NSEQ
        # broadcast the two sums rows over the head-pair's partition halves
        pb = ctx.prb.tile([P, 512], F32, tag="prb", name="pb")
        nc.tensor.matmul(pb[0:64, :NSEQ], W["ones"][0:1, 0:64],
                         smb[0:1, 0:NSEQ], start=True, stop=True)
        nc.tensor.matmul(pb[64:128, :NSEQ], W["ones"][0:1, 0:64],
                         smb[0:1, 256:256 + NSEQ], start=True, stop=True)
        po = ctx.pp.tile([P, 512], F32, tag="pp", name="po")
        for hi in range(2):
            h = 2 * hp + hi
            ob = 64 * hi
            for kt, (kb, kp) in enumerate(kts):
                nc.tensor.matmul(po[ob:ob + 64, :NSEQ],
                                 vts[2 * j + kt][:kp, h * HD:(h + 1) * HD],
                                 aes[hi][:kp, kt, :], start=(kt == 0),
                                 stop=(kt == 1))
        rbs = ctx.rbs.tile([P, NSEQ], F32, tag="rbs")
        nc.vector.reciprocal(rbs[:], pb[:, :NSEQ])
        nc.vector.tensor_tensor(oT[:, hp, c0:c0 + NSEQ], po[:, :NSEQ],
                                rbs[:], op=OP.mult)

    prev = None
    for j in range(2):  # seq in pair
        c0 = j * NSEQ
        for hp in range(DK):  # head pair = feature chunk
            aes = []
            for hi in range(2):
                h = 2 * hp + hi
                qof = 64 * (h % 2)
                qch, kch = h // 2, DK + h // 2
                q = qkT[qof:qof + 64, qch, c0:c0 + NSEQ]
                sT = ctx.psb.tile([P, 2, 256], F32, tag="psb", name="sT")
                for kt, (kb, kp) in enumerate(kts):
                    nc.tensor.matmul(sT[:kp, kt, :NSEQ],
                                     qkT[qof:qof + 64, kch, c0 + kb:c0 + kb + kp],
                                     q, start=True, stop=True)
                ae = ctx.ae.tile([P, 2, NSEQ], BF, tag="ae")
                # one exp over both key-chunks (rows 69:128 of chunk 1 hold
                # stale finite psum values; they are never read downstream)
                nc.scalar.activation(ae[:, :, :], sT[:, :, :NSEQ], AF.Exp)
                aes.append(ae)
            sm = ctx.prb.tile([P, 512], F32, tag="prb", name="sm")
            for hi in range(2):
                cb = 256 * hi
                for kt, (kb, kp) in enumerate(kts):
                    nc.tensor.matmul(sm[0:1, cb:cb + NSEQ], W["ones"][:kp, 0:1],
                                     aes[hi][:kp, kt, :], start=(kt == 0),
                                     stop=(kt == 1))
            smb = ctx.smb.tile([1, 453], BF, tag="smb")
            nc.scalar.activation(smb[0:1, :], sm[0:1, 0:453], AF.Identity)
            if prev is not None:
                phase2(prev)
            prev = (j, hp, aes, smb)
    phase2(prev)
    return oT


def emit_matmul_fm(ctx, nc, wname, kn, inT, combine):
    """dense feature-major matmul: out[:, mc, :] for mc in range(6)."""
    w = ctx.W[wname]
    for mc in range(DK):
        ps = ctx.pmm.tile([P, 512], F32, tag="mm", name="mmps")
        ps = ps[:, :TAU]
        for k in range(kn):
            nc.tensor.matmul(ps[:], w[:, k, mc * P:(mc + 1) * P], inT[:, k, :],
                             start=(k == 0), stop=(k == kn - 1))
        combine(mc, ps)


def emit_delta_add(ctx, nc, deltaT, xts, tiles):
    """transpose feature-major delta and accumulate into token-major xts;
    all 6 feature chunks of a token tile share one psum bank so the
    accumulate is a single wide vector op."""
    W = ctx.W
    for i, (r0, pi, co) in enumerate(tiles):
        tp = ctx.pp.tile([P, 8, P], BF, tag="pp", name="tpd")
        for j in range(DK):
            nc.tensor.transpose(tp[:pi, j, :], deltaT[:, j, co:co + pi],
                                W["ident"][:, :])
        nc.vector.tensor_tensor(xts[i][:pi], xts[i][:pi],
                                tp[:pi, 0:DK, :], op=OP.add)


def emit_pair_gen(ctx, nc, d, branch, rowbase):
    W = ctx.W
    tiles = PAIR_TILES
    # ---- stage A: load + LN1
    xts = []
    for (r0, pi, co) in tiles:
        xt = ctx.xres.tile([P, DK, P], F32, tag="xres")
        nc.sync.dma_start(xt[:pi], d["x"][bass.ds(rowbase + r0, pi), :, :])
        xts.append(xt)
    xnT = emit_ln(ctx, nc, xts, tiles)
    yield

    # ---- branch-specific pre-attention
    if branch == "T":
        aT = ctx.fmB.tile([P, DK, TAU], BF, tag="fmB")

        def tab_comb(mc, ps):
            nc.scalar.activation(aT[:, mc, :], ps[:], AF.Identity,
                                 bias=W["btabu"][:, mc:mc + 1])
        emit_adapter(ctx, nc, "tab", xnT, tab_comb)
        attn_in = aT
        saT = None
    else:
        saT = ctx.sa.tile([P, DK, TAU], BF, tag="saT")

        def sa_comb(mc, ps):
            nc.scalar.activation(saT[:, mc, :], ps[:], AF.Identity,
                                 bias=W["bsau"][:, mc:mc + 1])
        emit_adapter(ctx, nc, "sa", xnT, sa_comb)
        attn_in = xnT
    yield

    # ---- attention
    oT = emit_attention(ctx, nc, d, attn_in, tiles)
    yield

    # ---- proj (+ branch combine) -> delta1
    delta1 = ctx.fmC.tile([P, DK, TAU], BF, tag="fmC")
    if branch == "T":
        attnT = ctx.fmB.tile([P, DK, TAU], BF, tag="fmB")

        def proj_comb(mc, ps):
            nc.scalar.activation(attnT[:, mc, :], ps[:], AF.Identity,
                                 bias=W["bproj"][:, mc:mc + 1])
        emit_matmul_fm(ctx, nc, "wproj", DK, oT, proj_comb)

        def ta_comb(mc, ps):
            nc.scalar.activation(delta1[:, mc, :], ps[:], AF.Identity,
                                 bias=W["btau"][:, mc:mc + 1])
        emit_adapter(ctx, nc, "ta", attnT, ta_comb)
    else:
        def proj_comb_s(mc, ps):
            nc.vector.scalar_tensor_tensor(delta1[:, mc, :], ps[:],
                                           W["bproj"][:, mc:mc + 1],
                                           saT[:, mc, :],
                                           op0=OP.add, op1=OP.add)
        emit_matmul_fm(ctx, nc, "wproj", DK, oT, proj_comb_s)

    # ---- first residual: x2 = x + delta1 (in-place on xts)
    emit_delta_add(ctx, nc, delta1, xts, tiles)
    yield

    # ---- LN2 (+ sm adapter for spatial); MLP runs fused across streams
    if branch == "T":
        # prefetch the first fc1 double-chunks while the scalar DMA queue
        # is otherwise idle, so the fused MLP doesn't start cold
        ctx.wf1_pre = []
        for ocp in range(2):
            wt = ctx.wf1.tile([P, 2 * DK * P], BF, tag="wf1", name="wf1pre")
            nc.scalar.dma_start(wt[:], d["wfc1"][ocp])
            ctx.wf1_pre.append(wt)
    xn2T = emit_ln(ctx, nc, xts, tiles)
    smT = None
    if branch == "S":
        smT = ctx.sa.tile([P, DK, TAU], BF, tag="saT")

        def sm_comb(mc, ps):
            nc.scalar.activation(smT[:, mc, :], ps[:], AF.Identity,
                                 bias=W["bsmu"][:, mc:mc + 1])
        emit_adapter(ctx, nc, "sm", xn2T, sm_comb)
    yield {"branch": branch, "rowbase": rowbase, "xts": xts,
           "xn2T": xn2T, "smT": smT}


def emit_mlp_fused(ctx, nc, d, sts):
    """fc1/fc2 over both streams with each weight chunk streamed once."""
    W = ctx.W
    tiles = PAIR_TILES
    g2s = [ctx.g2.tile([P, HK, TAU], BF, tag="g2", name="g2")
           for _ in sts]
    for ocp in range(HK // 2):
        if ocp < len(ctx.wf1_pre):
            wt = ctx.wf1_pre[ocp]
        else:
            wt = ctx.wf1.tile([P, 2 * DK * P], BF, tag="wf1")
            nc.scalar.dma_start(wt[:], d["wfc1"][ocp])
        for sub in range(2):
            oc = 2 * ocp + sub
            for st, g2 in zip(sts, g2s):
                ps = ctx.pmm.tile([P, 512], F32, tag="mm", name="mmps")
                ps = ps[:, :TAU]
                for k in range(DK):
                    nc.tensor.matmul(ps[:], wt[:, (sub * DK + k) * P:
                                                (sub * DK + k + 1) * P],
                                     st["xn2T"][:, k, :], start=(k == 0),
                                     stop=(k == DK - 1))
                nc.scalar.activation(g2[:, oc, :], ps[:],
                                     AF.Gelu_apprx_sigmoid,
                                     bias=W["bfc1s"][:, oc:oc + 1])
    mlpTs = {}
    deltas = {}
    for st in sts:
        deltas[st["branch"]] = ctx.fmC.tile([P, DK, TAU], BF,
                                            tag="fmC", name="delta2")
        if st["branch"] == "T":
            mlpTs["T"] = ctx.fmB.tile([P, DK, TAU], BF, tag="fmB",
                                      name="mlpT")
    for mc in range(DK):
        pss = []
        for st in sts:
            psf = ctx.pmm.tile([P, 512], F32, tag="mm", name="mmps")
            pss.append(psf[:, :TAU])
        for half in range(2):
            wt = ctx.wf2.tile([P, HK * P // 2], BF, tag="wf2")
            nc.scalar.dma_start(wt[:], d["wfc2"][mc, :,
                                                 half * HK * P // 2:
                                                 (half + 1) * HK * P // 2])
            for st, g2, ps in zip(sts, g2s, pss):
                for k in range(HK // 2):
                    nc.tensor.matmul(ps[:], wt[:, k * P:(k + 1) * P],
                                     g2[:, HK // 2 * half + k, :],
                                     start=(half == 0 and k == 0),
                                     stop=(half == 1 and k == HK // 2 - 1))
        for st, ps in zip(sts, pss):
            if st["branch"] == "T":
                nc.scalar.activation(mlpTs["T"][:, mc, :], ps[:], AF.Identity,
                                     bias=W["bfc2"][:, mc:mc + 1])
            else:
                nc.vector.scalar_tensor_tensor(deltas["S"][:, mc, :], ps[:],
                                               W["bfc2"][:, mc:mc + 1],
                                               st["smT"][:, mc, :],
                                               op0=OP.add, op1=OP.add)
    for st in sts:
        if st["branch"] == "T":
            def tm_comb(mc, ps):
                nc.scalar.activation(deltas["T"][:, mc, :], ps[:], AF.Identity,
                                     bias=W["btmu"][:, mc:mc + 1])
            emit_adapter(ctx, nc, "tm", mlpTs["T"], tm_comb)
    for st in sts:
        emit_delta_add(ctx, nc, deltas[st["branch"]], st["xts"], tiles)
        for i, (r0, pi, co) in enumerate(tiles):
            nc.scalar.dma_start(d["y"][bass.ds(st["rowbase"] + r0, pi), :, :],
                                st["xts"][i][:pi])


def build_program(npairs=4, loop=True, reps=1, unroll=4):
    import contextlib
    nc = bacc.Bacc("TRN2", target_bir_lowering=False, debug=False,
                   num_devices=NCORES)
    d = {}
    d["x"] = nc.dram_tensor("x", [ROWS, DK, P], F32, kind="ExternalInput").ap()
    for name, shape, dt in WEIGHT_SPECS + STREAMED_SPECS:
        d[name] = nc.dram_tensor(name, shape, dt, kind="ExternalInput").ap()
    d["y"] = nc.dram_tensor("y", [ROWS, DK, P], F32, kind="ExternalOutput").ap()

    with tile.TileContext(nc) as tc:
        with contextlib.ExitStack() as es:
            ctx = Ctx()
            make_pools(ctx, tc, es)
            load_weights(ctx, nc, d)

            def body_pairgroup(i):
                # interleave the two independent pair pipelines stage-by-stage
                # so each stream's serial dependency chains are filled with the
                # other stream's engine work; the MLP runs fused across both
                # streams so each fc weight chunk is streamed once per group
                gens = [emit_pair_gen(ctx, nc, d, "T", i),
                        emit_pair_gen(ctx, nc, d, "S", i + TT * NSEQ)]
                alive = list(gens)
                sts = []
                while alive:
                    for g in list(alive):
                        try:
                            st = next(g)
                            if st is not None:
                                sts.append(st)
                        except StopIteration:
                            alive.remove(g)
                emit_mlp_fused(ctx, nc, d, sts)

            def body_all():
                if loop:
                    with tc.For_i(0, npairs * TAU, unroll * TAU,
                                  staggered_reset=True) as i:
                        for u in range(unroll):
                            body_pairgroup(i + u * TAU)
                else:
                    for p in range(npairs):
                        body_pairgroup(p * TAU)

            if reps > 1:
                with tc.For_i(0, reps, 1):
                    body_all()
            else:
                body_all()
            if os.environ.get("KERNEL_POOL_REPORT"):
                for nm in dir(ctx):
                    p = getattr(ctx, nm)
                    if isinstance(p, tile.TilePool):
                        print("pool %-10s %8.2f KB/part  space=%s"
                              % (nm, p.current_size() / (1024 * P), p.space))
    nc.compile()
    return nc


# ----------------------------------------------------------------------------
# harness entry point
# ----------------------------------------------------------------------------

_CACHED = {}


def kernel(**inputs):
    if "nc" not in _CACHED:
        _CACHED["nc"] = build_program()
    nc = _CACHED["nc"]
    w = preprocess_weights(inputs)
    x = np.asarray(inputs["x"], np.float32)  # [128, 197, 768]
    in_maps = []
    for c in range(NCORES):
        m = dict(w)
        m["x"] = np.ascontiguousarray(
            x[c * T:(c + 1) * T].reshape(ROWS, DK, P))
        in_maps.append(m)
    res = run_bass_kernel_spmd(nc, in_maps, core_ids=list(range(NCORES)))
    out = np.stack([r["y"].reshape(T, NSEQ, D) for r in res.results])
    return out.reshape(NCORES * T, NSEQ, D)

